# revision 32
# baseline (speedup 1.0000x reference)
"""Trainium2 Bass kernel for nn_MultiHeadAttention_30846455119878.

8-core strategy:
  - Attention is head-sharded: core m owns heads {2m, 2m+1}; it computes
    q/k/v projections for its 2 heads over all B*T tokens, then causal
    softmax attention per (batch, head).
  - The output projection contracts over ALL heads, so each batch's attention
    features ([128 feats x T]) are re-sharded heads->tokens with a small bf16
    AllToAll; each core then runs the full output projection for its 1/8
    token slice locally (contraction over all 1024 features) plus bias.
  - Host side: x is passed pre-transposed as x^T [C, B*T] twice: bf16 (for
    the v projection) and fp8-e4m3 (for q/k); Wq/Wk are fp8 with a x64 host
    prescale (their ~0.02 magnitudes would be e4m3 subnormals), compensated
    in the exp scale.  All other matmuls are bf16 x bf16 -> fp32 PSUM.

v3 performance state (HW, 8 axon trn2 cores): ~365-410us depending on the
machine's power-throttle state, best observed 364.4us (v1 baseline:
461-468us); rel err 6.4e-3.
Run-to-run variance is large (same binary measured 364..465us): the chip
spends most of the run power-throttled to 13/16 or 4/8 of 2.4GHz
(throttle_avg_util_limit ~0.70), and entry skew makes each AllToAll cost
8-35us.  Compare kernels by the min of >=3 runs.

Structure (the load-bearing decisions, all HW-verified):
  - Software-pipelined phases: proj(b+1) chunks are emitted inside
    attention(b)'s query-chunk loop.  Attention is ScalarE(exp)-bound
    (~36us/batch), projections are PE-bound; overlapped they hide each other
    and keep the PE dense (HAM warm).
  - Both heads' scores land in one 2-bank PSUM tile [128, 2, 512] -> ONE exp
    ACTIVATE per key-block (halves ScalarE instruction overhead); mask via
    two DVE muls against an fp8 0/1 mask.
  - fp8 DoubleRow (contraction 256/matmul) for the q/k projections: the
    score error is crushed by the 1/sqrt(C) scale + exp, so e4m3 is safe
    here (adds ~2.4e-3 rel err).  NOT safe for v / att / outproj: their fp8
    error passes straight into the output (~3.6-5%, gate is 2e-2).
  - Per-query-chunk normalize + staging: den reciprocal bounces via a
    [128, 8] reshape DMA (a [1, 1024] single-partition DVE reciprocal is
    ~7.8us (!) vs ~150ns for the 128-lane form), then rb-broadcast matmul +
    DVE mul + 4 cc_in stores, all overlapped under later attention chunks.
  - Queue map (critical): sync = xt/weight loads + den/rec resheshapes + tail
    rcv preloads; scalar = exp ONLY (plus startup loads and tail stores);
    gpsimd = cc_in staging + collectives + wo/bo + outproj(3) DMAs;
    tensor/vector = compute only.  Two hard-won rules:
      (1) a collective_compute's queue entry BLOCKS its engine queue until
          the collective completes -> nothing schedule-critical may sit
          behind a cc on gpsimd;
      (2) PE MATMULs issue strictly in program order -> a matmul whose dep
          resolves late (e.g. outproj on a pending rcv) dams every later
          matmul; only emit outproj where its rcv is already resolvable.
      Also (3): tile-pool rings are FIFO per tag -> an early-emitted tile
          allocation that waits on a late dep head-of-line blocks later
          allocations of the same tag (keep outproj psA tiles behind the
          rb tiles of the current batch, never ahead of proj chunks).
  - Tail: outproj(1) rcv preloaded on sync, half its matmuls fill the last
    normalize chain, the rest + all of outproj(2) execute inside cc(3)'s
    shadow; 24 dummy matmuls bridge PE warmth through the collective so
    outproj(3) runs at full clock.  Final stores are quarter-split across
    two DMA queues (a single out-store tile drains at ~26GB/s/engine).
    Den-row PSUM copies are emitted before the att_un copies (the
    reciprocal chain is the per-chunk critical path).
  - Startup is pure HBM bandwidth (~6.5MB before full speed): fp8 x copies
    land first so all q/k projections run from ~2us; 18 warmup matmuls on a
    resident tile trip the HAM clock gate off the 1.2GHz cold state.

Dead ends (HW-measured, do not retry):
  - outproj emitted right after cc(b) or at batch start: PE-FIFO dams /
    psA ring head-of-line (447-492us regressions).
  - per-tcb [1, 1024] DVE reciprocal: 575us.
  - den/rec DMAs on gpsimd (blocked by cc) or mid-batch on sync before the
    xt prefetch existed: 5-14us PE stalls per batch.
  - v-proj via wv-stationary matmuls + PE transposes: a wash (transpose
    ~275ns each, PE_SBUF_ACCESS_LATENCY-dominated, and doesn't warm HAM).
  - fp8 DoubleRow for v (needs an a+b residual pass, 2x matmuls: net loss);
    fp8 anywhere post-softmax: precision.
  - splitting any batch's AllToAll: per-op floor + queue-blocking eat the
    gain (8 collectives measured 564us in v1).
  - deriving batch-0's fp8 x by DVE-casting the bf16 chunks (to trim the
    startup stream): the cast needs the FULL bf16 chunk, so q/k lose their
    early fp8 start and the PE starves 10-28us (min 384-390 vs 372-378 for
    the straight fp8 DMA).  Hybrid (DMA 2 chunks, cast 2) also worse.
  - reciprocal_approx_fast / gpsimd partition_broadcast: BROKEN in this
    runtime; XBAR dma_start_transpose shears with strided src; max 1
    sync-wait per instruction -> bacc.Bacc.
"""

import sys

if "/opt/trn_rl_repo" not in sys.path:
    sys.path.insert(0, "/opt/trn_rl_repo")

import numpy as np
import ml_dtypes

import concourse.bass as bass
import concourse.tile as tile
from concourse import bacc, mybir
from concourse.bass_utils import run_bass_kernel_spmd
from concourse.tile_rust import add_dep_helper

BF16 = ml_dtypes.bfloat16
F8E4 = ml_dtypes.float8_e4m3fn

# Full problem dims
B_FULL, T_FULL, C_FULL, H_FULL, D_HEAD = 4, 2048, 1024, 16, 64
N_CORES = 8
HPC = H_FULL // N_CORES  # heads per core = 2
F = HPC * D_HEAD         # per-core attention feature rows = 128
TCH = 512                # query-chunk (free dim of score matmuls)
D = D_HEAD


def build_nc(B=B_FULL, T=T_FULL, C=C_FULL):
    """Build the SPMD Bass graph (same graph on all 8 cores)."""
    dt = mybir.dt
    CK = C // 128        # contraction chunks for projections
    NTC = T // TCH       # query chunks per sequence
    NSB = T // 128       # key blocks per sequence
    SBB = TCH // 128     # key blocks that overlap one query chunk diagonal = 4
    TS = T // N_CORES    # token shard per (batch, core) = 256
    CO = H_FULL * D_HEAD  # output feature dim (Wo cols) = 1024
    TT = 128             # token tile for output projection
    SLOTS = NTC * HPC    # denominator slots per batch = 8
    scale = float(1.0 / np.sqrt(C))
    scale8 = scale / 4096.0

    nc = bacc.Bacc()
    xt_d = nc.declare_dram_parameter("xt", [128, CK, B * T], dt.bfloat16, isOutput=False)
    xt8_d = nc.declare_dram_parameter("xt8", [128, CK, B * T], dt.float8e4, isOutput=False)
    wq8_d = nc.declare_dram_parameter("wq8", [128, CK, F], dt.float8e4, isOutput=False)
    wk8_d = nc.declare_dram_parameter("wk8", [128, CK, F], dt.float8e4, isOutput=False)
    wv_d = nc.declare_dram_parameter("wv", [128, CK, F], dt.bfloat16, isOutput=False)
    wo_d = nc.declare_dram_parameter("wo", [128, N_CORES, CO], dt.bfloat16, isOutput=False)
    bo_d = nc.declare_dram_parameter("bo", [1, CO], dt.bfloat16, isOutput=False)
    mask_d = nc.declare_dram_parameter("mask", [128, SBB, TCH], dt.float8e4, isOutput=False)
    out_d = nc.declare_dram_parameter("out", [B, TS, CO], dt.float32, isOutput=True)

    cc_in = [nc.dram_tensor(f"cc_in{b}", [N_CORES, F, TS], dt.bfloat16) for b in range(B)]
    cc_out = [nc.dram_tensor(f"cc_out{b}", [N_CORES, F, TS], dt.bfloat16) for b in range(B)]
    rg = [list(range(N_CORES))]

    with tile.TileContext(nc) as tc:
        from contextlib import ExitStack

        with ExitStack() as ctx:
            wpool = ctx.enter_context(tc.tile_pool(name="w", bufs=1))
            xpool = ctx.enter_context(tc.tile_pool(name="xt", bufs=4))
            x8pool = ctx.enter_context(tc.tile_pool(name="x8", bufs=4))
            qkpool = ctx.enter_context(tc.tile_pool(name="qk", bufs=2))
            v1pool = ctx.enter_context(tc.tile_pool(name="v1", bufs=2))
            epool = ctx.enter_context(tc.tile_pool(name="exp", bufs=8))
            apool = ctx.enter_context(tc.tile_pool(name="attn", bufs=4))
            recpool = ctx.enter_context(tc.tile_pool(name="rec", bufs=2))
            aupool = ctx.enter_context(tc.tile_pool(name="attu", bufs=2))
            denpool = ctx.enter_context(tc.tile_pool(name="den", bufs=2))
            rcvpool = ctx.enter_context(tc.tile_pool(name="rcv", bufs=4))
            outpool = ctx.enter_context(tc.tile_pool(name="osb", bufs=2))
            psA = ctx.enter_context(tc.tile_pool(name="psA", bufs=2, space="PSUM"))
            psS = ctx.enter_context(tc.tile_pool(name="psS", bufs=2, space="PSUM"))
            psB = ctx.enter_context(tc.tile_pool(name="psB", bufs=2, space="PSUM"))

            # resident constants; xt chunk DMAs are issued by proj chunks on
            # the same sync queue.  wo/bo (2MB, needed only at outproj) go on
            # the gpsimd queue so they don't delay the first projections.
            wq_sb = wpool.tile([128, CK, F], dt.float8e4, tag="wq")
            wk_sb = wpool.tile([128, CK, F], dt.float8e4, tag="wk")
            wv_sb = wpool.tile([128, CK, F], dt.bfloat16, tag="wv")
            wo_sb = wpool.tile([128, N_CORES, CO], dt.bfloat16, tag="wo")
            bo_sb = wpool.tile([1, CO], dt.bfloat16, tag="bo")
            mask_sb = wpool.tile([128, SBB, TCH], dt.float8e4, tag="mask")
            ones_sb = wpool.tile([D + 1, 128], dt.bfloat16, tag="ones")
            nc.sync.dma_start(out=wq_sb, in_=wq8_d[:, :, :])
            nc.scalar.dma_start(out=wk_sb, in_=wk8_d[:, :, :])
            nc.vector.memset(ones_sb, 1.0)
            # PE warmup: ~4us of dummy matmuls while the startup DMAs stream,
            # so the HAM clock gate reaches 8/8 before the first projection
            warm_ps = psA.tile([128, TCH], dt.float32, tag="mm", name="warm_ps")
            for w in range(18):
                nc.tensor.matmul(
                    warm_ps[0:D + 1, 0:128],
                    lhsT=ones_sb[:, 0:D + 1], rhs=ones_sb[:, :],
                    start=(w == 0), stop=(w == 17),
                )

            # -------- per-batch projection state and one-chunk emitter ------
            def new_proj_state(b):
                qT = qkpool.tile([F, T], dt.bfloat16, tag="qT", name=f"qT_{b}")
                kT = qkpool.tile([F, T], dt.bfloat16, tag="kT", name=f"kT_{b}")
                v1 = v1pool.tile([128, NSB, HPC, 80], dt.bfloat16, tag="v1", name=f"v1_{b}")
                nc.vector.memset(v1[:, :, :, D:D + 1], 1.0)
                return {"qT": qT, "kT": kT, "v1": v1}

            def load_xt(b, tcb, xt_eng=None, split=False):
                g0 = b * T + tcb * TCH
                eng = xt_eng or nc.sync
                x8_sb = x8pool.tile([128, CK, TCH], dt.float8e4, tag="x8",
                                    name=f"x8_{b}_{tcb}")
                xt_sb = xpool.tile([128, CK, TCH], dt.bfloat16, tag="xt",
                                   name=f"xt_{b}_{tcb}")
                if split:
                    # fp8 copy first (q/k run first), then per-CK bf16
                    # sub-loads: lets the first matmuls start ~3us into the
                    # startup DMA stream instead of after the full chunk
                    eng.dma_start(out=x8_sb, in_=xt8_d[:, :, g0:g0 + TCH])
                    for o in range(CK):
                        eng.dma_start(
                            out=xt_sb[:, o, :], in_=xt_d[:, o, g0:g0 + TCH])
                else:
                    eng.dma_start(out=x8_sb, in_=xt8_d[:, :, g0:g0 + TCH])
                    eng.dma_start(out=xt_sb, in_=xt_d[:, :, g0:g0 + TCH])
                return (x8_sb, xt_sb)

            def emit_proj_chunk(b, tcb, st, xt_pre=None, parts="qkv"):
                x8_sb, xt_sb = xt_pre if xt_pre is not None else load_xt(b, tcb)
                qk_w = ((wq_sb, st["qT"]), (wk_sb, st["kT"])) if "qk" in parts else ()
                for w_sb, dstT in qk_w:
                    ps = psA.tile([128, TCH], dt.float32, tag="mm")
                    for o2 in range(CK // 2):
                        nc.tensor.matmul(
                            ps,
                            lhsT=w_sb[:, 2 * o2:2 * o2 + 2, :],
                            rhs=x8_sb[:, 2 * o2:2 * o2 + 2, :],
                            start=(o2 == 0), stop=(o2 == CK // 2 - 1),
                            perf_mode=mybir.MatmulPerfMode.DoubleRow,
                        )
                    nc.vector.tensor_copy(
                        out=dstT[:, tcb * TCH:(tcb + 1) * TCH], in_=ps
                    )
                # v directly in [s, d] layout: v[s, f] = sum_c x[s, c] Wv[c, f]
                for ssub in (range(SBB) if "v" in parts else ()):
                    vps_full = psA.tile([128, TCH], dt.float32, tag="mm",
                                        name=f"vps_{b}_{tcb}_{ssub}")
                    vps = vps_full[:, 0:F]
                    for o in range(CK):
                        nc.tensor.matmul(
                            vps,
                            lhsT=xt_sb[:, o, ssub * 128:(ssub + 1) * 128],
                            rhs=wv_sb[:, o, :],
                            start=(o == 0), stop=(o == CK - 1),
                        )
                    stx = tcb * SBB + ssub
                    for h in range(HPC):
                        nc.vector.tensor_copy(
                            out=st["v1"][:, stx, h, 0:D], in_=vps[:, h * D:(h + 1) * D]
                        )

            # -------- output projection emitter (per batch) -----------------
            def emit_outproj_rcv(b, eng):
                rcv = rcvpool.tile([128, N_CORES, TS], dt.bfloat16, tag="rcv",
                                   name=f"rcv_{b}")
                for j in range(N_CORES):
                    rcv_rd = eng.dma_start(out=rcv[:, j, :], in_=cc_out[b][j, :, :])
                    add_dep_helper(rcv_rd.ins, cc_insts[b], sync=True,
                                   reason="cc_out RAW")
                return rcv

            def emit_outproj_mms(b, rcv, eng, groups, eng2=None):
                for gi, (tt, c2) in enumerate(groups):
                    seng = eng2 if (eng2 is not None and gi % 2) else eng
                    ps = psA.tile([128, TCH], dt.float32, tag="mm",
                                  name=f"ops_{b}_{tt}_{c2}")
                    for j in range(N_CORES):
                        nc.tensor.matmul(
                            ps[0:TT, 0:512],
                            lhsT=rcv[:, j, tt * TT:(tt + 1) * TT],
                            rhs=wo_sb[:, j, c2 * 512:(c2 + 1) * 512],
                            start=(j == 0), stop=False,
                        )
                    nc.tensor.matmul(
                        ps[0:TT, 0:512],
                        lhsT=ones_sb[0:1, 0:TT],
                        rhs=bo_sb[0:1, c2 * 512:(c2 + 1) * 512],
                        start=False, stop=True,
                    )
                    osb = outpool.tile([TT, 512], dt.float32, tag="osb",
                                       name=f"osb_{b}_{tt}_{c2}")
                    nc.vector.tensor_copy(out=osb, in_=ps[0:TT, 0:512])
                    # split each store across engines: a single out-store
                    # tile drains at only ~26GB/s/engine
                    oeng = eng2 if eng2 is not None else seng
                    nq = 4 if eng2 is not None else 2
                    qs = TT // nq
                    for qi in range(nq):
                        (seng if qi % 2 == 0 else oeng).dma_start(
                            out=out_d[b, tt * TT + qi * qs:tt * TT + (qi + 1) * qs,
                                      c2 * 512:(c2 + 1) * 512],
                            in_=osb[qi * qs:(qi + 1) * qs, :],
                        )

            ALL_GROUPS = [(tt, c2) for tt in range(TS // TT)
                          for c2 in range(CO // 512)]

            def emit_outproj(b, eng=None):
                eng = eng if eng is not None else nc.gpsimd
                rcv = emit_outproj_rcv(b, eng)
                emit_outproj_mms(b, rcv, eng, ALL_GROUPS)

            # ---------------- main pipelined batch loop ---------------------
            cc_insts = []
            st = [None] * B
            st[0] = new_proj_state(0)
            # batch-0 prelude: startup is pure HBM bandwidth.  The fp8 x
            # copies (0.5MB/chunk) land first so ALL q/k projections can run
            # back-to-back from ~2us; the bf16 copies for the v projections
            # stream behind them across both DMA queues.
            xt_pre0 = []
            for tcb in range(NTC):
                eng = nc.scalar if tcb % 2 else nc.sync
                x8_sb = x8pool.tile([128, CK, TCH], dt.float8e4, tag="x8",
                                    name=f"x8_0_{tcb}")
                eng.dma_start(out=x8_sb, in_=xt8_d[:, :, tcb * TCH:(tcb + 1) * TCH])
                xt_pre0.append(x8_sb)
            nc.sync.dma_start(out=wv_sb, in_=wv_d[:, :, :])
            nc.scalar.dma_start(out=mask_sb, in_=mask_d[:, :, :])
            xt_full0 = []
            for tcb in range(NTC):
                eng = nc.scalar if tcb % 2 else nc.sync
                xt_sb = xpool.tile([128, CK, TCH], dt.bfloat16, tag="xt",
                                   name=f"xt_0_{tcb}")
                # per-ssub sub-loads: each v matmul group consumes one
                # 128-token block, so v(0) starts ~7us earlier than waiting
                # for the whole 1MB chunk
                for ss in range(SBB):
                    eng.dma_start(
                        out=xt_sb[:, :, ss * 128:(ss + 1) * 128],
                        in_=xt_d[:, :, tcb * TCH + ss * 128:tcb * TCH + (ss + 1) * 128])
                xt_full0.append(xt_sb)
            for tcb in range(NTC):
                emit_proj_chunk(0, tcb, st[0],
                                xt_pre=(xt_pre0[tcb], xt_full0[tcb]), parts="qk")
            for tcb in range(NTC):
                emit_proj_chunk(0, tcb, st[0],
                                xt_pre=(xt_pre0[tcb], xt_full0[tcb]), parts="v")

            for b in range(B):
                if b + 1 < B:
                    st[b + 1] = new_proj_state(b + 1)
                qT, kT, v1 = st[b]["qT"], st[b]["kT"], st[b]["v1"]

                # ---- causal attention, both heads in one exp/mask op
                attn_h = [apool.tile([D, T], dt.bfloat16, tag="attn",
                                     name=f"attn_{b}_{hh}") for hh in range(HPC)]
                att_un = aupool.tile([D, SLOTS, TCH], dt.bfloat16, tag="attu",
                                     name=f"attu_{b}")
                den_b = denpool.tile([1, SLOTS * TCH], dt.bfloat16, tag="den",
                                     name=f"den_{b}")
                rec_all = recpool.tile([1, SLOTS * TCH], dt.bfloat16,
                                       tag="recall", name=f"recall_{b}")
                # prefetch all of next batch's input up-front on the sync
                # queue: the den/rec reshape DMAs below then never delay an
                # xt load (and vice versa nothing here waits on a collective)
                xt_next = ([load_xt(b + 1, t) for t in range(NTC)]
                           if b + 1 < B else None)
                stg_insts = []
                for tcb in range(NTC):
                    att_ps = [psB.tile([D + 1, TCH], dt.float32, tag="att",
                                       name=f"attps_{b}_{tcb}_{hh}")
                              for hh in range(HPC)]
                    nsb = SBB * (tcb + 1)
                    for sb in range(nsb):
                        j0 = sb - SBB * tcb
                        # columns t < j0*128 of this (key-block, query-chunk)
                        # pair are fully causal-masked -> skipped everywhere
                        c0 = j0 * 128 if j0 > 0 else 0
                        sps = psS.tile([128, HPC, TCH], dt.float32, tag="sps",
                                       name=f"sps_{b}_{tcb}_{sb}")
                        for h in range(HPC):
                            nc.tensor.matmul(
                                sps[:, h, c0:TCH],
                                lhsT=kT[h * D:(h + 1) * D, sb * 128:(sb + 1) * 128],
                                rhs=qT[h * D:(h + 1) * D, tcb * TCH + c0:(tcb + 1) * TCH],
                                start=True, stop=True,
                                tile_position=(h * D, 0),
                            )
                        et = epool.tile([128, HPC, TCH], dt.bfloat16, tag="exp",
                                        name=f"et_{b}_{tcb}_{sb}")
                        nc.scalar.activation(
                            out=et[:, :, c0:TCH], in_=sps[:, :, c0:TCH],
                            func=mybir.ActivationFunctionType.Exp, scale=scale8,
                        )
                        if j0 >= 0:
                            for h in range(HPC):
                                nc.vector.tensor_mul(
                                    et[:, h, c0:TCH], et[:, h, c0:TCH],
                                    mask_sb[:, j0, c0:TCH],
                                )
                        for h in range(HPC):
                            nc.tensor.matmul(
                                att_ps[h][:, c0:TCH],
                                lhsT=v1[:, sb, h, 0:D + 1], rhs=et[:, h, c0:TCH],
                                start=(sb == 0), stop=(sb == nsb - 1),
                            )
                    for h in range(HPC):
                        slot = tcb * HPC + h
                        # denominator rows first: the reciprocal chain they
                        # feed is the per-chunk critical path
                        nc.vector.tensor_copy(
                            out=den_b[0:1, slot * TCH:(slot + 1) * TCH],
                            in_=att_ps[h][D:D + 1, :],
                        )
                    for h in range(HPC):
                        slot = tcb * HPC + h
                        nc.vector.tensor_copy(out=att_un[:, slot, :], in_=att_ps[h][0:D, :])
                    # overlap next batch's projections with this attention
                    if b + 1 < B:
                        emit_proj_chunk(b + 1, tcb, st[b + 1],
                                        xt_pre=xt_next[tcb])
                    # half of outproj(B-3) fills the PE bubble while the last
                    # chunk's reciprocal chain runs (it is already runnable,
                    # and small enough not to delay cc(B-1))
                    if b == B - 1 and tcb == 3:
                        emit_outproj_mms(B - 3, rcv_31, nc.sync, ALL_GROUPS[:2])
                    # outproj(b-2) mid-attention: its rcv resolved when
                    # cc(b-1) completed, so the matmuls (strict PE program
                    # order!) never dam up the queue, and they fill the
                    # ScalarE-bound window's PE slack.  For the last batch,
                    # outproj(B-3)/(B-2) are instead deferred into the tail to
                    # cover cc(B-1); only their rcv loads are issued here (on
                    # the idle sync queue).
                    if tcb == 2 and 2 <= b < B - 1:
                        emit_outproj(b - 2)
                    if b == B - 1 and tcb == 1:
                        rcv_31 = emit_outproj_rcv(B - 3, nc.sync)
                    if b == B - 1 and tcb == 3:
                        rcv_pen = emit_outproj_rcv(B - 2, nc.sync)
                    # per-chunk normalize + staging: the reciprocal / rb
                    # broadcast / multiply / cc_in stores for this query chunk
                    # overlap the remaining attention chunks, so only the last
                    # chunk's short chain is exposed before cc(b).  The recip
                    # bounces through a [128, 8] reshape DMA: a [1, 1024]
                    # single-partition reciprocal measures ~7.8us on HW, the
                    # 128-lane one ~150ns.
                    den_t = recpool.tile([128, HPC * TCH // 128], dt.bfloat16,
                                         tag="dent", name=f"dent_{b}_{tcb}")
                    nc.sync.dma_start(
                        out=den_t,
                        in_=den_b[0:1, tcb * HPC * TCH:(tcb + 1) * HPC * TCH])
                    rec_t = recpool.tile([128, HPC * TCH // 128], dt.bfloat16,
                                         tag="rect", name=f"rect_{b}_{tcb}")
                    with nc.allow_low_precision(reason="bf16 softmax denom recip is plenty at rel-err 2e-2"):
                        nc.vector.reciprocal(out=rec_t, in_=den_t)
                    nc.sync.dma_start(
                        out=rec_all[0:1, tcb * HPC * TCH:(tcb + 1) * HPC * TCH],
                        in_=rec_t)
                    for h in range(HPC):
                        slot = tcb * HPC + h
                        rb_ps = psA.tile([D, TCH], dt.float32, tag="mm",
                                         name=f"rb_{b}_{slot}")
                        nc.tensor.matmul(
                            rb_ps, lhsT=ones_sb[0:1, 0:D],
                            rhs=rec_all[0:1, slot * TCH:(slot + 1) * TCH],
                            start=True, stop=True,
                        )
                        nc.vector.tensor_mul(
                            attn_h[h][:, tcb * TCH:(tcb + 1) * TCH],
                            att_un[:, slot, :], rb_ps,
                        )
                    for h in range(HPC):
                        for j in (2 * tcb, 2 * tcb + 1):
                            stg_insts.append(nc.gpsimd.dma_start(
                                out=cc_in[b][j, h * D:(h + 1) * D, :],
                                in_=attn_h[h][:, j * TS:(j + 1) * TS],
                            ).ins)

                if b == 0:
                    # wo/bo needed first at outproj(0) (~cc(0) done); loading
                    # them here keeps 2.5MB of HBM traffic out of the startup
                    # projection's critical path.
                    nc.gpsimd.dma_start(out=wo_sb, in_=wo_d[:, :, :])
                    nc.gpsimd.dma_start(out=bo_sb, in_=bo_d[:, :])
                cc = nc.gpsimd.collective_compute(
                    "AllToAll", mybir.AluOpType.bypass, replica_groups=rg,
                    ins=[cc_in[b].ap().opt()], outs=[cc_out[b].ap().opt()],
                )
                for s in stg_insts:
                    add_dep_helper(cc.ins, s, sync=True, reason="cc_in RAW")
                cc_insts.append(cc.ins)


            # tail: outproj(B-3) second half + all of outproj(B-2) are ready
            # work that the PE FIFO reaches only after cc(B-1) is triggered --
            # they execute inside the collective's shadow (stores on sync/
            # scalar; gpsimd is blocked until the collective completes).
            # outproj(B-1) follows once cc(B-1) lands.
            emit_outproj_mms(B - 3, rcv_31, nc.sync, ALL_GROUPS[2:])
            emit_outproj_mms(B - 2, rcv_pen, nc.scalar, ALL_GROUPS,
                             eng2=nc.sync)
            warm2 = psA.tile([128, TCH], dt.float32, tag="mm", name="warm2")
            for w in range(24):
                nc.tensor.matmul(
                    warm2[0:D + 1, 0:128],
                    lhsT=ones_sb[:, 0:D + 1], rhs=ones_sb[:, :],
                    start=(w == 0), stop=(w == 23),
                )
            rcv_last = emit_outproj_rcv(B - 1, nc.gpsimd)
            emit_outproj_mms(B - 1, rcv_last, nc.gpsimd, ALL_GROUPS,
                             eng2=nc.scalar)

    nc.finalize()
    return nc


def prep_inputs(x, Wq, Wk, Wv, Wo, bo):
    """Host-side shard/layout prep. Returns in_maps for the 8 cores."""
    B, T, C = x.shape
    CK = C // 128
    SBB = TCH // 128

    x = np.asarray(x, dtype=np.float32)
    xTf = np.ascontiguousarray(x.reshape(B * T, C).T)              # [C, B*T] fp32
    xt = np.ascontiguousarray(
        xTf.astype(BF16).reshape(CK, 128, B * T).transpose(1, 0, 2))
    xt8 = np.ascontiguousarray(
        np.clip(xTf, -240, 240).astype(F8E4)
        .reshape(CK, 128, B * T).transpose(1, 0, 2))

    CO = Wo.shape[1]
    wo_h = np.ascontiguousarray(
        np.asarray(Wo, np.float32).astype(BF16).reshape(N_CORES, 128, CO).transpose(1, 0, 2)
    )
    bo_h = np.asarray(bo, np.float32).astype(BF16).reshape(1, CO)

    p = np.arange(128)[:, None, None]
    j = np.arange(SBB)[None, :, None]
    t = np.arange(TCH)[None, None, :]
    mask_h = (t >= p + j * 128).astype(F8E4)          # [128, SBB, TCH]

    in_maps = []
    for m in range(N_CORES):
        maps = {"xt": xt, "xt8": xt8, "wo": wo_h, "bo": bo_h, "mask": mask_h}
        for name, W in (("wq8", Wq), ("wk8", Wk), ("wv", Wv)):
            Ws = np.concatenate(
                [np.asarray(W[HPC * m + i], np.float32) for i in range(HPC)], axis=1
            )  # [C, F]
            if name == "wv":
                maps[name] = np.ascontiguousarray(
                    Ws.astype(BF16).reshape(CK, 128, F).transpose(1, 0, 2))
            else:
                # x64 prescale keeps the ~0.02-magnitude weights out of
                # e4m3's subnormal range; compensated in the exp scale
                maps[name] = np.ascontiguousarray(
                    np.clip(Ws * 64.0, -240, 240).astype(F8E4)
                    .reshape(CK, 128, F).transpose(1, 0, 2))
        in_maps.append(maps)
    return in_maps


_NC_CACHE = {}


def _get_nc(B, T, C):
    key = (B, T, C)
    if key not in _NC_CACHE:
        _NC_CACHE[key] = build_nc(B, T, C)
    return _NC_CACHE[key]


def kernel(x, Wq, Wk, Wv, Wo, bo, _trace=False):
    x = np.asarray(x)
    B, T, C = x.shape
    nc = _get_nc(B, T, C)
    in_maps = prep_inputs(x, Wq, Wk, Wv, Wo, bo)
    res = run_bass_kernel_spmd(
        nc, in_maps, core_ids=list(range(N_CORES)), trace=_trace
    )
    TS = T // N_CORES
    CO = np.asarray(Wo).shape[1]
    out = np.empty((B, T, CO), dtype=np.float32)
    for m in range(N_CORES):
        out[:, m * TS:(m + 1) * TS, :] = res.results[m]["out"]
    if _trace:
        kernel.last_result = res
    return out
